# revision 20
# baseline (speedup 1.0000x reference)
"""Trainium2 Bass kernel for a 6-layer dense transformer encoder.

Model: V=32000, D=768, H=12 heads (DH=64), FF=3072, L=6 layers, B=16, S=512.

Sharding: pure data-parallel over batch — 2 batches per NeuronCore x 8 cores,
no collectives. Each core runs the full encoder on its 1024 tokens.

Layout strategy (per core):
  - Activations live feature-major ("xT": [d on partitions, t on free]) so every
    projection matmul uses natural-layout weights (lhsT = W[d, e], rhs = xT).
  - V is computed token-major into an AUGMENTED layout: per head pair the
    columns are [v_even(64) | keep(1) | v_odd(64)] (129 cols, 774 total). The
    AV matmul then uses a 65-column lhsT so its PSUM output carries the
    keep-masked softmax denominator as an extra row — no separate denominator
    matmuls.
  - Attention logits are computed transposed (logitsT[k, q]) so exp(logits)
    lands directly in the [k, q] layout the AV matmul needs.
  - Softmax normalize: reciprocal_approx_fast (DVE, ~5x faster than
    reciprocal) + gpsimd partition_broadcast (PE does no broadcast matmuls).
  - No max-subtraction in softmax: logits are O(1) here, exp cannot overflow.
  - Biases: zero matmuls. bv is folded host-side into bo (bo' = bo + bv@wo);
    wo/FFN2 residual adds use scalar_tensor_tensor (ps + bias_col) + x.
    bq/bk/b1 ride the PSUM->SBUF copy ops.
  - LayerNorm: 4-chunk (256 token) software pipeline. Stats on the PE
    (ones-column matmuls, sum + sum-of-squares sharing one PSUM bank), row
    math fused to 2 scalar_tensor_tensor + 1 scalar Rsqrt + 1 mean scale,
    mean/rstd broadcast over partitions on gpsimd.
  - FFN: c2-outer (512-token chunk), FFN2 accumulates all 24 k-chunks of a
    chunk directly in PSUM (k-outer over 6 concurrent et banks) — one fused
    residual+bias DVE op per (et, chunk) instead of per-fc adds. ft is built
    in 2 half-passes of 12 k-chunks to halve its SBUF footprint.

dtypes: bf16 matmul operands, fp32 PSUM accumulation, fp32 trunk for
residuals/LN stats (stats matmuls use fp32r bitcast).
"""

import os
import sys
from contextlib import ExitStack

import numpy as np

for _p in ("/opt/trn_rl_repo",):
    if _p not in sys.path and os.path.isdir(_p):
        sys.path.insert(0, _p)

import ml_dtypes  # noqa: E402

import concourse.bass as bass  # noqa: E402
import concourse.bacc as bacc  # noqa: E402
import concourse.tile as tile  # noqa: E402
from concourse import mybir  # noqa: E402

# ---------------------------------------------------------------- constants
V, D, H, FF, L = 32000, 768, 12, 3072, 6
B, S = 16, 512
DH = D // H              # 64
NCORES = 8
BL = B // NCORES         # 2 batches per core
T = BL * S               # 1024 tokens per core
P = 128
DT = D // P              # 6 feature tiles
TT = T // P              # 8 token tiles
FT = FF // P             # 24 ff tiles
KT = S // P              # 4 key tiles per batch
EPS = 1e-6
SQRTD = float(np.sqrt(float(D)))
INV_SQRT_DH = 1.0 / float(np.sqrt(float(DH)))
HP = H // 2              # 6 head pairs
VW = DH * 2 + 1          # 129 cols per head pair in augmented V
VA = HP * VW             # 774

F32 = mybir.dt.float32
F32R = mybir.dt.float32r
BF16 = mybir.dt.bfloat16
I32 = mybir.dt.int32
AF = mybir.ActivationFunctionType
ALU = mybir.AluOpType

LNC = 4                  # LN chunks
CW = T // LNC            # 256


def _pos_encoding_np():
    pos = np.arange(S, dtype=np.float64)[:, None]
    i = np.arange(D)[None, :]
    rates = 1.0 / np.power(10000.0, (2.0 * (i // 2).astype(np.float64)) / D)
    ang = pos * rates
    pe = np.where(i % 2 == 0, np.sin(ang), np.cos(ang))
    return pe.astype(np.float32)  # [S, D]


def build(nc: bass.Bass):
    """Declare DRAM I/O and trace the Tile program. SPMD: same program on all
    cores; only the `tokens` input differs per core."""
    tokens_d = nc.dram_tensor("tokens", [P, TT], I32, kind="ExternalInput")
    emb_d = nc.dram_tensor("emb", [V, D], F32R, kind="ExternalInput")
    posT_d = nc.dram_tensor("posT", [P, DT, S], F32, kind="ExternalInput")
    idn_d = nc.dram_tensor("idn", [P, P], F32R, kind="ExternalInput")
    onesc_d = nc.dram_tensor("onesc", [P, 1], F32R, kind="ExternalInput")

    drams = {}
    for n, sh, dt in [("wq", [L, D, D], BF16), ("wk", [L, D, D], BF16),
                      ("wv", [L, D, D], BF16), ("wo", [L, D, D], BF16),
                      ("w1", [L, D, FF], BF16), ("w2", [L, FF, D], BF16),
                      ("bq", [L, P, DT], F32), ("bk", [L, P, DT], F32),
                      ("bo", [L, P, DT], F32), ("b2", [L, P, DT], F32),
                      ("b1", [L, P, FT], F32),
                      ("g1", [L, P, DT], F32), ("be1", [L, P, DT], F32),
                      ("g2", [L, P, DT], F32), ("be2", [L, P, DT], F32)]:
        drams[n] = nc.dram_tensor(n, sh, dt, kind="ExternalInput")

    out_d = nc.dram_tensor("out", [T, D], F32, kind="ExternalOutput")

    with tile.TileContext(nc) as tc, ExitStack() as ctx:
        pools = {}

        def pool(name, bufs, space="SBUF"):
            pools[name] = ctx.enter_context(
                tc.tile_pool(name=name, bufs=bufs, space=space))
            return pools[name]

        # pools needed during embedding
        parp = pool("parp", 2)
        trunk = pool("trunk", 2)      # f32 [P, DT, T]
        ps_qk = pool("ps_qk", 2, space="PSUM")
        ps_lg = pool("ps_lg", 2, space="PSUM")
        ps_o = pool("ps_o", 4, space="PSUM")

        # ---------------- constants
        onesc = parp.tile([P, 1], F32R, tag="onesc", bufs=1)
        nc.sync.dma_start(onesc[:], onesc_d[:])
        idn = parp.tile([P, P], F32R, tag="idn", bufs=1)
        nc.sync.dma_start(idn[:], idn_d[:])

        tok = parp.tile([P, TT], I32, tag="tok", bufs=1)
        nc.sync.dma_start(tok[:], tokens_d[:])
        keep = parp.tile([P, TT], F32, tag="keep", bufs=1)
        nc.vector.tensor_scalar(out=keep[:], in0=tok[:], scalar1=0,
                                scalar2=None, op0=ALU.not_equal)
        keepb = parp.tile([P, TT], BF16, tag="keepb", bufs=1)
        nc.vector.tensor_copy(keepb[:], keep[:])
        onesh = parp.tile([P, HP], BF16, tag="onesh", bufs=1)
        nc.vector.memset(onesh[:], 1.0)
        eps1 = parp.tile([1, 1], F32, tag="eps1", bufs=1)
        nc.vector.memset(eps1[:], EPS)
        pools.update(onesc=onesc, idn=idn, keep=keep, keepb=keepb,
                     onesh=onesh, eps1=eps1, ps_qk=ps_qk, ps_lg=ps_lg,
                     ps_o=ps_o)

        # ---------------- embedding: gather + transpose + scale + pos
        x = trunk.tile([P, DT, T], F32R, tag="trunk", name="x0")
        with tc.tile_pool(name="embp", bufs=2) as embp:
            posT = embp.tile([P, DT, S], F32, tag="posT", bufs=1)
            nc.sync.dma_start(posT[:], posT_d[:])
            for tt in range(TT):
                g = embp.tile([P, D], F32R, tag="gather")
                nc.gpsimd.indirect_dma_start(
                    out=g[:], out_offset=None, in_=emb_d[:],
                    in_offset=bass.IndirectOffsetOnAxis(ap=tok[:, tt:tt + 1], axis=0),
                )
                sp = (tt % (S // P)) * P  # position offset within the batch
                for dt in range(DT):
                    pst = ps_qk.tile([P, P], F32R, tag="mm")
                    # xT block = (g_block)^T  (emb pre-scaled by sqrt(D) on host)
                    nc.tensor.transpose(pst[:], g[:, dt * P:(dt + 1) * P], idn[:])
                    nc.vector.tensor_add(x[:, dt, tt * P:(tt + 1) * P],
                                         pst[:], posT[:, dt, sp:sp + P])

        # remaining pools (allocated after embp released)
        acts = pool("acts", 2)        # bf16 [P, DT, T]   {x_b16, x1_b16}
        pool("qkp", 4)                # bf16 [P, T]       {q, k per head pair}
        pool("vta", 1)                # bf16 [P, TT, VA]
        pool("opool", 1)              # bf16 [P, DT, T]
        pool("apool", 4)              # bf16 [P, KT, S]
        pool("wqk", 3)                # bf16 [P, DT, D]
        pool("w1p", 2)                # bf16 [P, DT, S]
        pool("w2p", 4)                # bf16 [P, D]
        pool("ftp", 1)                # bf16 [P, 4, S]
        pool("dbp", 4)                # f32 [P, S]
        pool("mrBp", 2)               # f32 [P, 2, CW]
        pool("sqp", 2)                # f32 [P, CW]
        pool("tmpp", 4)               # f32 [P, CW]
        pool("rowp", 1)               # f32 rows

        xb = acts.tile([P, DT, T], BF16, tag="acts", name="x0b")
        for dt in range(DT):
            nc.scalar.copy(xb[:, dt, :], x[:, dt, :])

        # ---------------- layers
        for l in range(L):
            with nc.named_scope(f"layer{l}"):
                x, xb = _layer(nc, tc, l, x, xb, pools, drams)

        # ---------------- output: transpose back to token-major
        with nc.named_scope("out"):
            for tt in range(TT):
                o = pools["dbp"].tile([P, S], F32, tag="db", name=f"ostg{tt}",
                                      bufs=4)
                o2 = pools["dbp"].tile([P, S], F32, tag="db", name=f"ostg2_{tt}",
                                       bufs=4)
                for dt in range(DT):
                    pst = ps_qk.tile([P, P], F32R, tag="mm")
                    nc.tensor.transpose(pst[:], x[:, dt, tt * P:(tt + 1) * P], idn[:])
                    dst = o if dt < 4 else o2
                    off = dt * P if dt < 4 else (dt - 4) * P
                    nc.vector.tensor_copy(dst[:, off:off + P], pst[:])
                nc.sync.dma_start(out_d[tt * P:(tt + 1) * P, 0:S], o[:, :])
                nc.sync.dma_start(out_d[tt * P:(tt + 1) * P, S:D], o2[:, 0:D - S])

    return nc


def _layernorm(nc, pools, xin, g_t, b_t, outs, uid):
    """LN over d (partitions) of xin [P, DT, T] (f32r), 4 chunks of 256
    tokens, software-pipelined: stats for chunk c+1 run on the PE while the
    row chain (DVE/scalar) and broadcast (gpsimd) of chunk c proceed; applies
    are interleaved two chunks behind so the PE never blocks on row math."""
    ps_qk, rowp, mrBp, sqp, tmpp = (pools["ps_qk"], pools["rowp"],
                                    pools["mrBp"], pools["sqp"], pools["tmpp"])
    onesc = pools["onesc"]
    D2 = float(D) * float(D)

    mbs = []

    def emit_stats_rows(c):
        cols = slice(c * CW, (c + 1) * CW)
        st = ps_qk.tile([1, 2, CW], F32, tag="mm", name=f"st{uid}_{c}")
        # squares first (split scalar/DVE) so they are in flight while the
        # PE runs the sum group
        sqs = []
        for dt in range(DT):
            sq = sqp.tile([P, CW], F32R, tag="sq", bufs=4)
            if dt % 2 == 0:
                nc.scalar.square(sq[:], xin[:, dt, cols])
            else:
                nc.vector.tensor_tensor(out=sq[:], in0=xin[:, dt, cols],
                                        in1=xin[:, dt, cols], op=ALU.mult)
            sqs.append(sq)
        for dt in range(DT):
            nc.tensor.matmul(st[:, 0, :], lhsT=onesc[:], rhs=xin[:, dt, cols],
                             start=(dt == 0), stop=(dt == DT - 1),
                             skip_group_check=True)
        for dt in range(DT):
            nc.tensor.matmul(st[:, 1, :], lhsT=onesc[:], rhs=sqs[dt][:],
                             start=(dt == 0), stop=(dt == DT - 1),
                             skip_group_check=True)
        # row chain: mean = u/D ; rstd = rsqrt(q/D - u^2/D^2 + eps)
        mr = rowp.tile([1, 2, CW], F32, tag="mr", name=f"mr{uid}_{c}", bufs=2)
        tp = rowp.tile([1, 2, CW], F32, tag="tp", name=f"tp{uid}_{c}", bufs=2)
        nc.vector.tensor_scalar(out=mr[:, 0, :], in0=st[:, 0, :],
                                scalar1=1.0 / float(D), scalar2=None,
                                op0=ALU.mult)
        nc.vector.tensor_tensor(out=tp[:, 0, :], in0=mr[:, 0, :],
                                in1=mr[:, 0, :], op=ALU.mult)
        nc.vector.scalar_tensor_tensor(
            out=tp[:, 1, :], in0=st[:, 1, :], scalar=1.0 / float(D),
            in1=tp[:, 0, :], op0=ALU.mult, op1=ALU.subtract)
        nc.scalar.activation(tp[:, 0, :], tp[:, 1, :], AF.Sqrt,
                             bias=pools["eps1"][:], scale=1.0)
        nc.vector.reciprocal_approx_fast(out=mr[:, 1, :], in_=tp[:, 0, :])
        mb = mrBp.tile([P, 2, CW], F32, tag="mrB", name=f"mb{uid}_{c}")
        nc.gpsimd.partition_broadcast(mb[:, 0, :], mr[:, 0, :])
        nc.gpsimd.partition_broadcast(mb[:, 1, :], mr[:, 1, :])
        mbs.append(mb)

    def emit_apply(c):
        cols = slice(c * CW, (c + 1) * CW)
        mb = mbs[c]
        for dt in range(DT):
            t1 = tmpp.tile([P, CW], F32R, tag="lt", name=f"l1{uid}_{c}_{dt}")
            nc.gpsimd.tensor_tensor(out=t1[:], in0=xin[:, dt, cols],
                                    in1=mb[:, 0, :], op=ALU.subtract)
            t2 = tmpp.tile([P, CW], F32R, tag="lt", name=f"l2{uid}_{c}_{dt}")
            nc.vector.tensor_tensor(out=t2[:], in0=t1[:], in1=mb[:, 1, :],
                                    op=ALU.mult)
            nc.vector.tensor_scalar(out=outs[0][:, dt, cols], in0=t2[:],
                                    scalar1=g_t[:, dt:dt + 1],
                                    scalar2=b_t[:, dt:dt + 1],
                                    op0=ALU.mult, op1=ALU.add)
            nc.scalar.activation(outs[1][:, dt, cols], t2[:], AF.Identity,
                                 bias=b_t[:, dt:dt + 1], scale=g_t[:, dt:dt + 1])

    # pipeline: stats0, stats1, apply0, stats2, apply1, stats3, apply2, apply3
    emit_stats_rows(0)
    emit_stats_rows(1)
    emit_apply(0)
    emit_stats_rows(2)
    emit_apply(1)
    emit_stats_rows(3)
    emit_apply(2)
    emit_apply(3)


def _layer(nc, tc, l, x, xb, pools, drams):
    trunk, acts, qkp = pools["trunk"], pools["acts"], pools["qkp"]
    vta_p, opool, apool = pools["vta"], pools["opool"], pools["apool"]
    wqk, w1p, w2p, ftp = pools["wqk"], pools["w1p"], pools["w2p"], pools["ftp"]
    dbp, rowp, parp = pools["dbp"], pools["rowp"], pools["parp"]
    ps_qk, ps_lg, ps_o = pools["ps_qk"], pools["ps_lg"], pools["ps_o"]
    keep, keepb, onesh = pools["keep"], pools["keepb"], pools["onesh"]

    # ---- per-layer params to SBUF
    par = {}
    for n in ("bq", "bk", "bo", "b2", "g1", "be1", "g2", "be2", "b1"):
        sh = [P, FT] if n == "b1" else [P, DT]
        t = parp.tile(sh, F32, tag=n, name=f"{n}{l}", bufs=2)
        nc.sync.dma_start(t[:], drams[n][l])
        par[n] = t

    def load_w_dd(name):
        w = wqk.tile([P, DT, D], BF16, tag="wqk", name=f"{name}{l}")
        nc.sync.dma_start(w[:], drams[name][l].rearrange("(a p) e -> p a e", p=P))
        return w

    # ================= attention =================
    # V projection (token-major, augmented [v_even | keep | v_odd] per pair),
    # masked rows zeroed via keep scale
    wv = load_w_dd("wv")
    vta = vta_p.tile([P, TT, VA], BF16, tag="vta", name=f"vta{l}")
    for tt in range(TT):
        # keep columns (shared by both heads of each pair), once per tt
        nc.vector.tensor_scalar(
            out=vta[:, tt, DH::VW], in0=onesh[:],
            scalar1=keep[:, tt:tt + 1], scalar2=None, op0=ALU.mult)
        for (c0, cn, npair) in ((0, S, 4), (S, D - S, 2)):
            ps = ps_qk.tile([P, cn], F32, tag="mm")
            for dt in range(DT):
                nc.tensor.matmul(ps[:], lhsT=xb[:, dt, tt * P:(tt + 1) * P],
                                 rhs=wv[:, dt, c0:c0 + cn],
                                 start=(dt == 0), stop=(dt == DT - 1))
            # strided write: feature f -> col (f//128)*129 + (f%128, +1 if >=64)
            pbase = c0 // P
            src = ps[:].rearrange("p (a h d) -> p a h d", h=2, d=DH)
            dst = vta[:, tt, pbase * VW:(pbase + npair) * VW].rearrange(
                "p (a w) -> p a w", w=VW)
            nc.vector.tensor_scalar(out=dst[:, :, 0:DH], in0=src[:, :, 0, :],
                                    scalar1=keep[:, tt:tt + 1],
                                    scalar2=None, op0=ALU.mult)
            nc.vector.tensor_scalar(out=dst[:, :, DH + 1:VW], in0=src[:, :, 1, :],
                                    scalar1=keep[:, tt:tt + 1],
                                    scalar2=None, op0=ALU.mult)

    wq = load_w_dd("wq")
    wk = load_w_dd("wk")
    # Flush of each (et, b) pair is emitted one pair later so the PE never
    # blocks on the DVE/gpsimd normalize chain.
    oT = opool.tile([P, DT, T], BF16, tag="oT", name=f"oT{l}")
    pending = []

    def flush_pending():
        pse_, pso_, et_, b_ = pending.pop(0)
        bcols_ = slice(b_ * S, (b_ + 1) * S)
        dne = rowp.tile([1, S], F32, tag="dn", name=f"dne{l}_{et_}_{b_}", bufs=2)
        dno = rowp.tile([1, S], F32, tag="dn", name=f"dno{l}_{et_}_{b_}", bufs=2)
        dst = rowp.tile([1, 2, S], F32, tag="dst", name=f"dst{l}_{et_}_{b_}",
                        bufs=1)
        nc.scalar.copy(dst[:, 0, :], pse_[DH:DH + 1, :])
        nc.scalar.copy(dst[:, 1, :], pse_[96:97, :])
        nc.vector.reciprocal_approx_fast(out=dne[:], in_=dst[:, 0, :])
        nc.vector.reciprocal_approx_fast(out=dno[:], in_=dst[:, 1, :])
        dbe = dbp.tile([P, S], F32, tag="db", name=f"dbe{l}_{et_}_{b_}")
        dbo = dbp.tile([P, S], F32, tag="db", name=f"dbo{l}_{et_}_{b_}")
        nc.gpsimd.partition_broadcast(dbe[:], dne[:])
        nc.gpsimd.partition_broadcast(dbo[:], dno[:])
        nc.vector.tensor_tensor(out=oT[0:DH, et_, bcols_],
                                in0=pse_[0:DH, :], in1=dbe[0:DH, :],
                                op=ALU.mult)
        nc.vector.tensor_tensor(out=oT[DH:P, et_, bcols_],
                                in0=pso_[DH:P, :], in1=dbo[DH:P, :],
                                op=ALU.mult)

    for et in range(DT):
        # Q/K projections for this head pair (feature-major; 1/sqrt(DH) in Q)
        qp = qkp.tile([P, T], BF16, tag="qk", name=f"q{l}_{et}")
        kp = qkp.tile([P, T], BF16, tag="qk", name=f"k{l}_{et}")
        for c2 in range(T // S):
            cols = slice(c2 * S, (c2 + 1) * S)
            psq = ps_qk.tile([P, S], F32, tag="mm")
            psk = ps_qk.tile([P, S], F32, tag="mm")
            for dt in range(DT):
                nc.tensor.matmul(psq[:], lhsT=wq[:, dt, et * P:(et + 1) * P],
                                 rhs=xb[:, dt, cols],
                                 start=(dt == 0), stop=(dt == DT - 1))
            for dt in range(DT):
                nc.tensor.matmul(psk[:], lhsT=wk[:, dt, et * P:(et + 1) * P],
                                 rhs=xb[:, dt, cols],
                                 start=(dt == 0), stop=(dt == DT - 1))
            nc.vector.tensor_scalar(out=qp[:, cols], in0=psq[:],
                                    scalar1=INV_SQRT_DH,
                                    scalar2=par["bq"][:, et:et + 1],
                                    op0=ALU.mult, op1=ALU.add)
            nc.vector.tensor_scalar(out=kp[:, cols], in0=psk[:],
                                    scalar1=par["bk"][:, et:et + 1],
                                    scalar2=None, op0=ALU.add)
        for b in range(BL):
            bcols = slice(b * S, (b + 1) * S)
            # even head: psum rows 0..64 (o rows 0-63, denom row 64)
            # odd head:  psum rows 63..127 (denom row 63, o rows 64-127)
            pse = ps_o.tile([P, S], F32, tag="o")
            pso = ps_o.tile([P, S], F32, tag="o")
            ats = []
            for sub in range(2):
                ats.append(apool.tile([P, KT, S], BF16, tag="at",
                                      name=f"at{l}_{b}_{2*et+sub}"))
            # logits: alternate row groups (sub0 rows 0-63, sub1 rows 64-127)
            for kt in range(KT):
                kcols = slice(b * S + kt * P, b * S + (kt + 1) * P)
                for sub in range(2):
                    prows = slice(sub * DH, (sub + 1) * DH)
                    psl = ps_lg.tile([P, S], F32, tag="lg")
                    nc.tensor.matmul(psl[:], lhsT=kp[prows, kcols],
                                     rhs=qp[prows, bcols],
                                     start=True, stop=True)
                    nc.scalar.activation(ats[sub][:, kt, :], psl[:], AF.Exp)
            # AV: even head uses a 65-col lhsT so its softmax denominator
            # rides along as PSUM row 64; odd head writes rows 64-127 and its
            # denominator accumulates into row 96 of the even tile.
            pb = et * VW
            for kt in range(KT):
                vtile = b * KT + kt
                nc.tensor.matmul(pse[0:DH + 1, :],
                                 lhsT=vta[:, vtile, pb:pb + DH + 1],
                                 rhs=ats[0][:, kt, :],
                                 start=(kt == 0), stop=(kt == KT - 1),
                                 skip_group_check=True)
                nc.tensor.matmul(pso[DH:P, :],
                                 lhsT=vta[:, vtile, pb + DH + 1:pb + VW],
                                 rhs=ats[1][:, kt, :],
                                 start=(kt == 0), stop=(kt == KT - 1),
                                 skip_group_check=True, tile_position=(0, DH))
                nc.tensor.matmul(pse[96:97, :],
                                 lhsT=keepb[:, vtile:vtile + 1],
                                 rhs=ats[1][:, kt, :],
                                 start=(kt == 0), stop=(kt == KT - 1),
                                 skip_group_check=True, tile_position=(0, 96))
            pending.append((pse, pso, et, b))
            if len(pending) > 1:
                flush_pending()
    # ---- wo projection + residual (fused bias via scalar_tensor_tensor)
    wo = load_w_dd("wo")
    xr = trunk.tile([P, DT, T], F32R, tag="trunk", name=f"xres{l}")
    for c2 in range(T // S):
        if c2 == 1:
            while pending:
                flush_pending()
        cols = slice(c2 * S, (c2 + 1) * S)
        for et in range(DT):
            if c2 == 0 and et == DT - 1:
                while pending:
                    flush_pending()
            ps = ps_qk.tile([P, S], F32, tag="mm")
            for dt in range(DT):
                nc.tensor.matmul(ps[:], lhsT=wo[:, dt, et * P:(et + 1) * P],
                                 rhs=oT[:, dt, cols],
                                 start=(dt == 0), stop=(dt == DT - 1))
            nc.vector.scalar_tensor_tensor(
                out=xr[:, et, cols], in0=ps[:], scalar=par["bo"][:, et:et + 1],
                in1=x[:, et, cols], op0=ALU.add, op1=ALU.add)

    # ---- LN1 -> x1 (f32 trunk) + x1 bf16
    x1 = trunk.tile([P, DT, T], F32R, tag="trunk", name=f"x1_{l}")
    x1b = acts.tile([P, DT, T], BF16, tag="acts", name=f"x1b{l}")
    _layernorm(nc, pools, xr, par["g1"], par["be1"], [x1, x1b], uid=f"{l}a")

    # ================= FFN =================
    # c2-outer; FFN2 accumulates all 24 k-chunks in PSUM (6 concurrent et
    # banks: 4 from ps_o + 2 from ps_lg), ft built in 2 half-passes.
    xr2 = trunk.tile([P, DT, T], F32R, tag="trunk", name=f"xres2_{l}")
    NFC = FF // S            # 6 passes, one w1 chunk (4 k-chunks) each
    KC = S // P              # 4 k-chunks per pass
    for c2 in range(T // S):
        cols = slice(c2 * S, (c2 + 1) * S)
        ps2 = []
        for et in range(DT):
            pool_ = ps_o if et < 4 else ps_lg
            ps2.append(pool_.tile([P, S], F32, tag="o" if et < 4 else "lg",
                                  name=f"ps2_{l}_{c2}_{et}"))
        for fca in range(NFC):
            ft = ftp.tile([P, KC, S], BF16, tag="ft", name=f"ft{l}_{c2}_{fca}")
            w1c = w1p.tile([P, DT, S], BF16, tag="w1c",
                           name=f"w1c{l}_{c2}_{fca}")
            nc.sync.dma_start(
                w1c[:],
                drams["w1"][l].rearrange("(a p) e -> p a e", p=P)[
                    :, :, fca * S:(fca + 1) * S])
            for m4 in range(KC):
                fi = fca * KC + m4
                ps = ps_qk.tile([P, S], F32, tag="mm")
                for dt in range(DT):
                    nc.tensor.matmul(ps[:], lhsT=w1c[:, dt, m4 * P:(m4 + 1) * P],
                                     rhs=x1b[:, dt, cols],
                                     start=(dt == 0), stop=(dt == DT - 1))
                nc.scalar.activation(ft[:, m4, :], ps[:],
                                     AF.Relu, bias=par["b1"][:, fi:fi + 1])
            for k4 in range(KC):
                kt = fca * KC + k4
                w2t = w2p.tile([P, D], BF16, tag="w2t", name=f"w2t{l}_{c2}_{kt}")
                nc.sync.dma_start(w2t[:], drams["w2"][l][kt * P:(kt + 1) * P, :])
                for et in range(DT):
                    nc.tensor.matmul(ps2[et][:], lhsT=w2t[:, et * P:(et + 1) * P],
                                     rhs=ft[:, k4, :],
                                     start=(kt == 0), stop=(kt == FT - 1),
                                     skip_group_check=True)
        for et in range(DT):
            nc.vector.scalar_tensor_tensor(
                out=xr2[:, et, cols], in0=ps2[et][:],
                scalar=par["b2"][:, et:et + 1], in1=x1[:, et, cols],
                op0=ALU.add, op1=ALU.add)

    # ---- LN2 -> next x (f32) + bf16
    xn = trunk.tile([P, DT, T], F32R, tag="trunk", name=f"xn{l}")
    xnb = acts.tile([P, DT, T], BF16, tag="acts", name=f"xnb{l}")
    _layernorm(nc, pools, xr2, par["g2"], par["be2"], [xn, xnb], uid=f"{l}b")
    return xn, xnb


# ------------------------------------------------------------------ host side
_BUILT = None


def _get_built():
    global _BUILT
    if _BUILT is None:
        nc = bacc.Bacc("TRN2", target_bir_lowering=False, debug=False,
                       num_devices=NCORES)
        build(nc)
        nc.compile()
        _BUILT = nc
    return _BUILT


def _pack_inputs(inputs):
    """Host-side prep: shard tokens, cast weights to bf16, pack params."""
    bf = ml_dtypes.bfloat16
    f32 = np.float32

    def npa(x, dt=None):
        a = np.asarray(x)
        return a.astype(dt) if dt is not None else a

    tokens = npa(inputs["tokens"]).astype(np.int32)          # [B, S]
    emb = npa(inputs["emb"], f32)

    pe = _pos_encoding_np()                                   # [S, D]
    # posT: [P, DT, S]  posT[p, dt, s] = pe[s, dt*128+p]
    posT = np.ascontiguousarray(pe.T.reshape(DT, P, S).transpose(1, 0, 2))

    def packP(a, ncol=DT):  # [L, X] -> [L, P, X/P]
        return np.ascontiguousarray(
            npa(a, f32).reshape(L, ncol, P).transpose(0, 2, 1))

    # fold V bias into wo bias: bo' = bo + bv @ wo  (per layer)
    bo_eff = npa(inputs["bo"], f32) + np.einsum(
        "ld,lde->le", npa(inputs["bv"], f32), npa(inputs["wo"], f32))

    shared = {
        "emb": emb * SQRTD, "posT": posT,
        "idn": np.eye(P, dtype=f32),
        "onesc": np.ones((P, 1), dtype=f32),
        "wq": npa(inputs["wq"]).astype(bf), "wk": npa(inputs["wk"]).astype(bf),
        "wv": npa(inputs["wv"]).astype(bf), "wo": npa(inputs["wo"]).astype(bf),
        "w1": npa(inputs["w1"]).astype(bf), "w2": npa(inputs["w2"]).astype(bf),
        "bq": packP(npa(inputs["bq"], f32) * INV_SQRT_DH),
        "bk": packP(inputs["bk"]),
        "bo": packP(bo_eff), "b2": packP(inputs["b2"]),
        "b1": packP(inputs["b1"], ncol=FT),
        "g1": packP(inputs["ln1_g"]), "be1": packP(inputs["ln1_b"]),
        "g2": packP(inputs["ln2_g"]), "be2": packP(inputs["ln2_b"]),
    }
    in_maps = []
    for c in range(NCORES):
        tc_ = tokens[c * BL:(c + 1) * BL].reshape(T)          # [1024]
        # [P, TT]: col tt, partition p -> token tt*P+p
        tok_tile = np.ascontiguousarray(tc_.reshape(TT, P).T)
        m = dict(shared)
        m["tokens"] = tok_tile
        in_maps.append(m)
    return in_maps


def kernel(**inputs) -> np.ndarray:
    from concourse.bass_utils import run_bass_kernel_spmd
    nc = _get_built()
    in_maps = _pack_inputs(inputs)
    res = run_bass_kernel_spmd(nc, in_maps, list(range(NCORES)))
    outs = [res.results[c]["out"].reshape(BL, S, D) for c in range(NCORES)]
    return np.concatenate(outs, axis=0).astype(np.float32)


if __name__ == "__main__":
    rng = np.random.default_rng(0)
    ins = {
        "tokens": rng.integers(0, V, (B, S)).astype(np.int32),
        "emb": rng.standard_normal((V, D), dtype=np.float32) * 0.02,
    }
    for n, sh in [("wq", (L, D, D)), ("wk", (L, D, D)), ("wv", (L, D, D)),
                  ("wo", (L, D, D)), ("w1", (L, D, FF)), ("w2", (L, FF, D))]:
        ins[n] = rng.standard_normal(sh, dtype=np.float32) * 0.02
    for n, sh in [("bq", (L, D)), ("bk", (L, D)), ("bv", (L, D)), ("bo", (L, D)),
                  ("b1", (L, FF)), ("b2", (L, D)),
                  ("ln1_b", (L, D)), ("ln2_b", (L, D))]:
        ins[n] = np.zeros(sh, np.float32)
    ins["ln1_g"] = np.ones((L, D), np.float32)
    ins["ln2_g"] = np.ones((L, D), np.float32)
    out = kernel(**ins)
    print(out.shape, out.dtype, np.abs(out).mean())
